# revision 28
# baseline (speedup 1.0000x reference)
"""Trainium2 Bass kernel for an 8x1024x768 pre-LN transformer encoder block.

Sharding: data-parallel over batch - 8 batch elements -> 8 NeuronCores, no
collectives. Each core runs the full block on its [1024, 768] slice.

v4 strategy (baseline v2 = 345us, v3 = 300us but over the error gate):
  - Attention kept in v2 shape: fp8-DoubleRow QKV/O projections, PE
    row-tiled fp8 scores, bf16 softmax/PV (dense PE work keeps the HAM
    clock-gate warm through the ACT-exp-bound attention phase).
  - FFN mixed precision: 1/3 of both FFN contractions run fp8e4 DoubleRow
    (w prescaled x16), the rest fp16 (free accuracy win over bf16).
    Halves a third of the dominant 123us FFN TensorE time while staying
    under the 2e-2 error gate (measured 1.5e-2 in simulation).
  - LN uses a single Rsqrt (folds sqrt+eps+reciprocal; eps contributes
    ~1e-5 relative, negligible).
  - Startup: x chunks stream on the sync+scalar HWDGE queues ahead of the
    attention weights, so the first LN chunks aren't stuck behind weight
    packets on the shared SDMA engines; w1(fp8 part)/wo prefetch during
    attention; outputs spread across three queues.
  - Matmul loops ordered so consecutive matmuls share the stationary
    operand (halves LDWEIGHTS traffic).

Residual stream, LN stats, PSUM accumulation stay fp32.
"""

import os

import numpy as np
import ml_dtypes

import concourse.bass as bass
import concourse.mybir as mybir
import concourse.tile as tile
from concourse import bacc
from concourse.bass_utils import run_bass_kernel_spmd
from concourse.masks import make_identity

P = 128
NT = 1024          # tokens per core
NI = NT // P       # 8 token chunks
D = 768
KC = D // P        # 6 feature chunks
KP = KC // 2       # 3 DoubleRow feature-chunk pairs
H = 12
DH = 64
F = 3072
MC = F // P        # 24 ffn chunks
EPS = 1e-5
WS = 16.0          # fp8 weight prescale
DR = mybir.MatmulPerfMode.DoubleRow

F1_8 = 2           # of KC=6 d_model chunks of FFN1 contraction in fp8-DR
F2_8 = 10          # of MC=24 d_ff chunks of FFN2 contraction in fp8-DR

f32 = mybir.dt.float32
bf16 = mybir.dt.bfloat16
f16 = mybir.dt.float16
f8 = mybir.dt.float8e4

_COMPILE_CACHE = {}
LAST_RESULT = None  # BassKernelResults of the most recent run (for test harness)


def _build(flags):
    has_bqk, has_bv, has_bo, has_b1, has_b2 = flags
    nc = bacc.Bacc("TRN2", target_bir_lowering=False, debug=False, num_devices=8)

    x_d = nc.dram_tensor("x", [NT, D], f32, kind="ExternalInput").ap()
    # fp8 attention weights, pre-reshaped host-side to [128, KC, D]:
    # element [p, kc, m] = scaled_w[kc*128 + p, m]
    wq_d = nc.dram_tensor("wq8", [P, KC, D], f8, kind="ExternalInput").ap()
    wk_d = nc.dram_tensor("wk8", [P, KC, D], f8, kind="ExternalInput").ap()
    wv_d = nc.dram_tensor("wv8", [P, KC, D], f8, kind="ExternalInput").ap()
    wo_d = nc.dram_tensor("wo8", [P, KC, D], f8, kind="ExternalInput").ap()
    # FFN weights, host-prescaled by 16: fp8 slabs for the DR chunks,
    # fp16 slabs for the rest.
    w18_d = nc.dram_tensor("w18", [P, F1_8, F], f8, kind="ExternalInput").ap()
    w1h_d = nc.dram_tensor("w1h", [P, KC - F1_8, F], f16, kind="ExternalInput").ap()
    w28_d = nc.dram_tensor("w28", [P, F2_8, D], f8, kind="ExternalInput").ap()
    w2h_d = nc.dram_tensor("w2h", [P, MC - F2_8, D], f16, kind="ExternalInput").ap()
    bq_d = nc.dram_tensor("bq", [D], f32, kind="ExternalInput").ap() if has_bqk else None
    bk_d = nc.dram_tensor("bk", [D], f32, kind="ExternalInput").ap() if has_bqk else None
    bv_d = nc.dram_tensor("bv", [D], f32, kind="ExternalInput").ap() if has_bv else None
    bo_d = nc.dram_tensor("bo", [D], f32, kind="ExternalInput").ap() if has_bo else None
    b1_d = nc.dram_tensor("b1", [F], f32, kind="ExternalInput").ap() if has_b1 else None
    b2_d = nc.dram_tensor("b2", [D], f32, kind="ExternalInput").ap() if has_b2 else None
    out_d = nc.dram_tensor("out", [NT, D], f32, kind="ExternalOutput").ap()

    with tile.TileContext(nc) as tc:
        sb = tc.alloc_tile_pool(name="sb", bufs=1, space="SBUF")
        ps = tc.alloc_tile_pool(name="ps", bufs=1, space="PSUM")

        # ---- constants ----
        ident = sb.tile([P, P], bf16, tag="ident", bufs=1, name="ident")
        make_identity(nc, ident)
        nb2 = sb.tile([P, 1], f32, tag="nb2", bufs=1, name="nb2")
        nc.vector.memset(nb2, -2.0)
        # 128*I in fp32: lets the O-proj residual ride the PSUM accumulator
        # (psum holds 128*(o + x) after one extra fp32 matmul).
        id128 = sb.tile([P, P], f32, tag="id128", bufs=1, name="id128")
        nc.gpsimd.memset(id128, 0.0)
        nc.gpsimd.affine_select(
            out=id128, in_=id128,
            compare_op=mybir.AluOpType.not_equal, fill=128.0,
            base=0, pattern=[[-1, P]], channel_multiplier=1,
        )

        def bcast_row(src_ap, n, name):
            t = sb.tile([P, n], f32, tag=name, bufs=1, name=name)
            nc.sync.dma_start(
                out=t,
                in_=bass.AP(
                    tensor=src_ap.tensor, offset=src_ap.offset, ap=[[0, P], [1, n]]
                ),
            )
            return t

        def chunk_vec(src_ap, nchunk, name):
            t = sb.tile([P, nchunk], f32, tag=name, bufs=1, name=name)
            nc.sync.dma_start(
                out=t,
                in_=bass.AP(
                    tensor=src_ap.tensor,
                    offset=src_ap.offset,
                    ap=[[1, P], [P, nchunk]],
                ),
            )
            return t

        bq_sb = chunk_vec(bq_d, KC, "bq_sb") if has_bqk else None
        bk_sb = chunk_vec(bk_d, KC, "bk_sb") if has_bqk else None
        b1_sb = chunk_vec(b1_d, MC, "b1_sb") if has_b1 else None
        bv_bc = bcast_row(bv_d, D, "bv_bc") if has_bv else None
        bo_bc = bcast_row(bo_d, D, "bo_bc") if has_bo else None
        b2_bc = bcast_row(b2_d, D, "b2_bc") if has_b2 else None

        # ---- persistent fp8 attention weights ----
        wq8 = sb.tile([P, KC, D], f8, tag="w8", bufs=4, name="wq8")
        wk8 = sb.tile([P, KC, D], f8, tag="w8", bufs=4, name="wk8")
        wv8 = sb.tile([P, KC, D], f8, tag="w8", bufs=4, name="wv8")
        wo8 = sb.tile([P, KC, D], f8, tag="w8", bufs=4, name="wo8")
        # persistent fp8 part of w1 (prefetched during attention); the
        # second "wf" slot later holds the fp8 part of w2.
        w18 = sb.tile([P, F1_8, F], f8, tag="wf", bufs=2, name="w18")

        # ---- persistent activations ----
        # Tag sharing (disjoint lifetimes -> same slots):
        #   bigA: xnT (LN1 -> last qk read)  /  hTh (FFN1 -> FFN2)
        #   bigB: v65 (LN1 -> last PV read)  /  hT8 (FFN1 -> FFN2)
        #   qkx:  qt,kt (-> last scores read) / xn2T8,xn2Th (LN2 -> FFN1)
        x_t = [sb.tile([P, D], f32, tag="x", bufs=NI, name=f"x{i}") for i in range(NI)]
        xnT = sb.tile([P, KC, NT], f8, tag="bigA", bufs=1, name="xnT")
        qt = sb.tile([P, KC, NT], f8, tag="qkx", bufs=2, name="qt")
        kt = sb.tile([P, KC, NT], f8, tag="qkx", bufs=2, name="kt")
        v65 = sb.tile([P, NI, 848], bf16, tag="bigB", bufs=1, name="v65")
        ot = sb.tile([P, KC, NT], f8, tag="ot", bufs=1, name="ot")

        def layernorm(src, dst, i):
            """dst = (src - mean(src)) / (std_unbiased(src) + eps), rowwise.

            Stats on DVE; the normalize itself runs on ACT as an affine
            Identity (scale=rstd, bias=-mean*rstd) so DVE can start the
            next chunk's stats in parallel.
            """
            st = sb.tile([P, 2, 6], f32, tag="stat", bufs=2, name=f"st{i}")
            xg = src.rearrange("p (s f) -> p s f", f=384)
            for s in range(2):
                nc.vector.bn_stats(out=st[:, s, :], in_=xg[:, s, :])
            mv = sb.tile([P, 2], f32, tag="mv", bufs=2, name=f"mv{i}")
            nc.vector.bn_aggr(out=mv, in_=st)
            sd = sb.tile([P, 1], f32, tag="sd", bufs=4, name=f"sd{i}")
            nc.scalar.activation(
                out=sd, in_=mv[:, 1:2], func=mybir.ActivationFunctionType.Sqrt,
                scale=float(D) / float(D - 1),
            )
            nc.vector.tensor_scalar_add(out=sd, in0=sd, scalar1=EPS)
            rstd = sb.tile([P, 1], f32, tag="sd", bufs=4, name=f"rstd{i}")
            nc.vector.reciprocal(out=rstd, in_=sd)
            nmr = sb.tile([P, 1], f32, tag="sd", bufs=4, name=f"nmr{i}")
            nc.vector.tensor_scalar(
                out=nmr, in0=mv[:, 0:1], scalar1=rstd, scalar2=-1.0,
                op0=mybir.AluOpType.mult, op1=mybir.AluOpType.mult,
            )
            nc.scalar.activation(
                out=dst, in_=src, func=mybir.ActivationFunctionType.Identity,
                scale=rstd, bias=nmr,
            )

        def transpose_chunks(xsrc, i, pfx):
            tp = ps.tile([P, D], bf16, tag="stp", bufs=3, name=f"{pfx}{i}")
            for k in range(KC):
                nc.tensor.transpose(
                    tp[:, k * P : (k + 1) * P], xsrc[:, k * P : (k + 1) * P], ident
                )
            return tp

        def v_proj(j):
            vaccs = [
                ps.tile([P, 512], f32, tag="smp", bufs=2, name=f"vps{j}_{hf}")
                for hf in range(2)
            ]
            for kp in range(KP):
                for hf in range(2):
                    nc.tensor.matmul(
                        vaccs[hf][:, 0:384],
                        xnT[:, 2 * kp : 2 * kp + 2, j * P : (j + 1) * P],
                        wv8[:, 2 * kp : 2 * kp + 2, hf * 384 : (hf + 1) * 384],
                        start=(kp == 0), stop=(kp == KP - 1),
                        perf_mode=DR,
                    )
            for hf in range(2):
                acc = vaccs[hf]
                vview = v65[:, j, hf * 390 : hf * 390 + 390].rearrange(
                    "p (h c) -> p h c", c=DH + 1
                )
                dst = vview[:, :, 0:DH]
                src = acc[:, 0:384].rearrange("p (h c) -> p h c", h=6)
                if has_bv:
                    nc.vector.tensor_add(
                        out=dst, in0=src,
                        in1=bv_bc[:, hf * 384 : (hf + 1) * 384].rearrange(
                            "p (h c) -> p h c", h=6
                        ),
                    )
                else:
                    nc.vector.tensor_copy(out=dst, in_=src)


        # ---- per token chunk: load, LN1, transpose, V projection ----
        # x chunks go FIRST on all three DMA-issue queues; the attention-
        # weight DMAs queue behind them (shared SDMA engines drain x first).
        # ones-column value 2.0: v65 holds 16*v, denominator row gets
        # 2*sum(p) -> normalize yields 8*attn (descale folded). Constant,
        # so written up front off the critical path.
        for j in range(NI):
            nc.vector.memset(
                v65[:, j, 0:780].rearrange("p (h c) -> p h c", c=DH + 1)[:, :, DH:],
                2.0,
            )
            nc.vector.memset(v65[:, j, 780:848], 0.0)

        for i in range(NI):
            q = (nc.sync, nc.scalar, nc.gpsimd)[i % 3]
            q.dma_start(out=x_t[i], in_=x_d[i * P : (i + 1) * P, :])
        nc.sync.dma_start(out=wv8, in_=wv_d)
        nc.scalar.dma_start(out=wk8, in_=wk_d)
        nc.scalar.dma_start(out=wq8, in_=wq_d)
        # PE warm-up: ~7us of dummy transposes while the x DMAs land, so
        # the HAM clock-gate is already at 8/8 when the real work starts.
        wrm = ps.tile([P, P], bf16, tag="smp", bufs=2, name="wrm")
        for _ in range(50):
            nc.tensor.transpose(wrm, ident, ident)

        for i in range(NI):
            xn = sb.tile([P, D], bf16, tag="xn", bufs=2, name=f"xn{i}")
            layernorm(x_t[i], xn, i)
            tp = transpose_chunks(xn, i, "tp")
            nc.scalar.copy(
                out=xnT[:, :, i * P : (i + 1) * P],
                in_=tp.rearrange("p (k c) -> p k c", k=KC),
            )
            v_proj(i)

        # ---- QK projections (fp8 DoubleRow) + row-tiled scores + bf16 PV ----
        EXP_SCALE = 0.125 / (WS * WS)

        # wo needed right after attention; w1's fp8 slab prefetched under
        # the attention phase (HBM is otherwise idle there).
        nc.sync.dma_start(out=wo8, in_=wo_d)
        nc.sync.dma_start(out=w18, in_=w18_d)

        def qk_proj(m, w8, b_sb, dstT, nm):
            acc = ps.tile([P, NT], f32, tag="stp", bufs=3, name=f"{nm}ps{m}")
            for kp in range(KP):
                for ih in range(2):
                    nc.tensor.matmul(
                        acc[:, ih * 512 : (ih + 1) * 512],
                        w8[:, 2 * kp : 2 * kp + 2, m * P : (m + 1) * P],
                        xnT[:, 2 * kp : 2 * kp + 2, ih * 512 : (ih + 1) * 512],
                        start=(kp == 0), stop=(kp == KP - 1),
                        perf_mode=DR,
                    )
            for ih in range(2):
                hsl = slice(ih * 512, (ih + 1) * 512)
                if has_bqk:
                    nc.vector.tensor_scalar_add(
                        out=dstT[:, m, hsl], in0=acc[:, hsl],
                        scalar1=b_sb[:, m : m + 1],
                    )
                else:
                    nc.vector.tensor_copy(out=dstT[:, m, hsl], in_=acc[:, hsl])

        def attn_scores_j(m, j, pts):
            # heads 2m (rows 0:64) and 2m+1 (rows 64:128), row-tiled pair.
            stps = [
                ps.tile([P, NT], f32, tag="stp", bufs=3, name=f"st{2 * m + hh}_{j}")
                for hh in range(2)
            ]
            for hh in range(2):
                r0, r1 = hh * DH, (hh + 1) * DH
                for ih in range(2):
                    nc.tensor.matmul(
                        stps[hh][:, ih * 512 : (ih + 1) * 512],
                        kt[r0:r1, m, j * P : (j + 1) * P],
                        qt[r0:r1, m, ih * 512 : (ih + 1) * 512],
                        start=True, stop=True,
                    )
            for hh in range(2):
                ptj = sb.tile(
                    [P, NT], bf16, tag="pt", bufs=16, name=f"pt{2 * m + hh}_{j}"
                )
                nc.scalar.activation(
                    out=ptj, in_=stps[hh],
                    func=mybir.ActivationFunctionType.Exp,
                    scale=EXP_SCALE, bias=nb2,
                )
                pts[hh].append(ptj)

        def attn_pv(h, pt_h):
            p_, hh = divmod(h, 2)
            r0, r1 = hh * DH, (hh + 1) * DH
            opv = [
                ps.tile([P, 512], f32, tag="smp", bufs=2, name=f"opv{h}_{iq}")
                for iq in range(2)
            ]
            for j in range(NI):
                for iq in range(2):
                    nc.tensor.matmul(
                        opv[iq],
                        v65[:, j, h * (DH + 1) : h * (DH + 1) + P],
                        pt_h[j][:, iq * 512 : (iq + 1) * 512],
                        start=(j == 0), stop=(j == NI - 1),
                    )
            for iq in range(2):
                ob = sb.tile([DH + 1, 512], f32, tag="ob", bufs=2, name=f"ob{h}_{iq}")
                nc.vector.tensor_copy(out=ob, in_=opv[iq][0 : DH + 1, :])
                dsb = sb.tile([1, 512], f32, tag="dsb", bufs=2, name=f"dsb{h}_{iq}")
                nc.vector.tensor_copy(out=dsb, in_=ob[DH : DH + 1, :])
                rc = sb.tile([1, 512], f32, tag="rc", bufs=2, name=f"rc{h}_{iq}")
                nc.vector.reciprocal_approx_fast(out=rc, in_=dsb)
                rb = sb.tile([DH, 512], f32, tag="rb", bufs=2, name=f"rb{h}_{iq}")
                nc.gpsimd.partition_broadcast(rb, rc)
                nc.vector.tensor_mul(
                    out=ot[r0:r1, p_, iq * 512 : (iq + 1) * 512],
                    in0=ob[0:DH, :], in1=rb,
                )

        # Software-pipelined attention: scores/exp stream continuously on
        # PE/ACT; the previous head-pair's PV and the next chunk's QK
        # projections are emitted into the gaps so the exp stream (the
        # phase's critical resource) never starves.
        qk_proj(0, wk8, bk_sb, kt, "k")
        qk_proj(0, wq8, bq_sb, qt, "q")
        prev = None
        for m in range(KC):
            pts = [[], []]
            for j in range(NI):
                attn_scores_j(m, j, pts)
                if j == 1 and prev is not None:
                    attn_pv(2 * prev[0], prev[1][0])
                if j == 3 and prev is not None:
                    attn_pv(2 * prev[0] + 1, prev[1][1])
                if j == 5 and m + 1 < KC:
                    qk_proj(m + 1, wk8, bk_sb, kt, "k")
                if j == 7 and m + 1 < KC:
                    qk_proj(m + 1, wq8, bq_sb, qt, "q")
            prev = (m, pts)
        attn_pv(2 * prev[0], prev[1][0])
        attn_pv(2 * prev[0] + 1, prev[1][1])

        # ---- O-proj (fp8 DoubleRow, psum = 128*o), residual, LN2, transpose ----
        xn2T8 = sb.tile([P, F1_8, NT], f8, tag="qkx", bufs=2, name="xn2T8")
        xn2Th = sb.tile([P, KC - F1_8, NT], f16, tag="qkx", bufs=2, name="xn2Th")
        for i in range(NI):
            oaccs = [
                ps.tile([P, 512], f32, tag="smp", bufs=2, name=f"ops{i}_{hf}")
                for hf in range(2)
            ]
            for cp in range(KP):
                for hf in range(2):
                    nc.tensor.matmul(
                        oaccs[hf][:, 0:384],
                        ot[:, 2 * cp : 2 * cp + 2, i * P : (i + 1) * P],
                        wo8[:, 2 * cp : 2 * cp + 2, hf * 384 : (hf + 1) * 384],
                        start=(cp == 0), stop=False,
                        perf_mode=DR,
                    )
            for hf in range(2):
                xsl = x_t[i][:, hf * 384 : (hf + 1) * 384]
                # psum += 128*x (8*WS == 128 folds the o descale), then ACT
                # evacuates x1 = x + o in one copy - no DVE work at all.
                nc.tensor.matmul(
                    oaccs[hf][:, 0:384], id128, xsl,
                    start=False, stop=True,
                )
                nc.scalar.activation(
                    out=xsl, in_=oaccs[hf][:, 0:384],
                    func=mybir.ActivationFunctionType.Identity,
                    scale=1.0 / (8.0 * WS),
                )
                if has_bo:
                    nc.vector.tensor_add(
                        out=xsl, in0=xsl, in1=bo_bc[:, hf * 384 : (hf + 1) * 384]
                    )
            xn2 = sb.tile([P, D], bf16, tag="xn", bufs=2, name=f"xn2_{i}")
            layernorm(x_t[i], xn2, NI + i)
            tq = transpose_chunks(xn2, i, "tq")
            nc.scalar.copy(
                out=xn2T8[:, :, i * P : (i + 1) * P],
                in_=tq[:, 0 : F1_8 * P].rearrange("p (k c) -> p k c", k=F1_8),
            )
            nc.scalar.copy(
                out=xn2Th[:, :, i * P : (i + 1) * P],
                in_=tq[:, F1_8 * P : D].rearrange("p (k c) -> p k c", k=KC - F1_8),
            )

        # ---- FFN1 mixed fp8-DR + fp16: acc = 16*(xn2 @ w1); gelu(acc/16) ----
        hT8 = sb.tile([P, F2_8, NT], f8, tag="bigB", bufs=1, name="hT8")
        hTh = sb.tile([P, MC - F2_8, NT], f16, tag="bigA", bufs=1, name="hTh")
        # persist w2 for FFN2: fp8 slab in the second "wf" slot; fp16 chunks
        # parked in the (now dead) softmax "pt" slots. FFN2 then has no DMA
        # dependence at all.
        w28p = sb.tile([P, F2_8, D], f8, tag="wf", bufs=2, name="w28p")
        nc.sync.dma_start(out=w28p, in_=w28_d)
        w2ht = [
            sb.tile([P, D], f16, tag="pt", bufs=16, name=f"w2h{k}")
            for k in range(MC - F2_8)
        ]
        for mp in range(MC // 2):
            w1hs = sb.tile([P, KC - F1_8, 2 * P], f16, tag="w1hs", bufs=3,
                           name=f"w1hs{mp}")
            nc.sync.dma_start(
                out=w1hs, in_=w1h_d[:, :, mp * 2 * P : (mp + 1) * 2 * P]
            )
            if mp >= 4:
                for t2 in range(2):
                    k = 2 * (mp - 4) + t2
                    if k < MC - F2_8:
                        nc.sync.dma_start(out=w2ht[k], in_=w2h_d[:, k, :])
            for t in range(2):
                m = 2 * mp + t
                acc = ps.tile([P, NT], f32, tag="stp", bufs=3, name=f"fps{m}")
                for ih in range(2):
                    nc.tensor.matmul(
                        acc[:, ih * 512 : (ih + 1) * 512],
                        w18[:, 0:F1_8, m * P : (m + 1) * P],
                        xn2T8[:, 0:F1_8, ih * 512 : (ih + 1) * 512],
                        start=True, stop=False,
                        perf_mode=DR,
                    )
                for c in range(KC - F1_8):
                    for ih in range(2):
                        nc.tensor.matmul(
                            acc[:, ih * 512 : (ih + 1) * 512],
                            w1hs[:, c, t * P : (t + 1) * P],
                            xn2Th[:, c, ih * 512 : (ih + 1) * 512],
                            start=False, stop=(c == KC - F1_8 - 1),
                        )
                hdst = hT8[:, m, :] if m < F2_8 else hTh[:, m - F2_8, :]
                nc.scalar.activation(
                    out=hdst, in_=acc,
                    func=mybir.ActivationFunctionType.Gelu,
                    scale=1.0 / WS,
                    bias=b1_sb[:, m : m + 1] if has_b1 else 0.0,
                )

        # ---- FFN2 mixed fp8-DR + fp16: per token half, 8 live accumulators ----
        for ihp in range(2):
            stps = [
                ps.tile([P, NT], f32, tag="stp", bufs=3, name=f"f2s{ihp}_{sl}")
                for sl in range(3)
            ]
            smps = [
                ps.tile([P, 512], f32, tag="smp", bufs=2, name=f"f2m{ihp}_{sl}")
                for sl in range(2)
            ]
            def accsl(il, dh):
                q = 2 * il + dh
                if q < 6:
                    return stps[q // 2][:, (q % 2) * 512 : (q % 2) * 512 + 384]
                return smps[q - 6][:, 0:384]
            # il-outer: ~40 consecutive matmuls per PSUM bank pair instead
            # of cycling all 8 banks every 8 matmuls (avoids the psum-queue
            # HAM oscillation), and lets each token chunk's residual start
            # while the next accumulates.
            NKH = MC - F2_8
            for il in range(4):
                tok = slice((4 * ihp + il) * P, (4 * ihp + il + 1) * P)
                for c in range(F2_8 // 2):
                    for dh_ in range(2):
                        nc.tensor.matmul(
                            accsl(il, dh_),
                            hT8[:, 2 * c : 2 * c + 2, tok],
                            w28p[:, 2 * c : 2 * c + 2, dh_ * 384 : (dh_ + 1) * 384],
                            start=(c == 0), stop=False,
                            perf_mode=DR,
                        )
                for kk in range(NKH):
                    for dh_ in range(2):
                        nc.tensor.matmul(
                            accsl(il, dh_),
                            hTh[:, kk, tok],
                            w2ht[kk][:, dh_ * 384 : (dh_ + 1) * 384],
                            start=False, stop=(kk == NKH - 1),
                        )
            for il in range(4):
                i = 4 * ihp + il
                for dh_ in range(2):
                    xsl = x_t[i][:, dh_ * 384 : (dh_ + 1) * 384]
                    nc.vector.scalar_tensor_tensor(
                        out=xsl, in0=accsl(il, dh_), scalar=1.0 / WS,
                        in1=xsl,
                        op0=mybir.AluOpType.mult, op1=mybir.AluOpType.add,
                    )
                    if has_b2:
                        nc.vector.tensor_add(
                            out=xsl, in0=xsl, in1=b2_bc[:, dh_ * 384 : (dh_ + 1) * 384]
                        )
                q = (nc.gpsimd, nc.sync, nc.scalar, nc.gpsimd)[il]
                q.dma_start(out=out_d[i * P : (i + 1) * P, :], in_=x_t[i])

        sb.release()
        ps.release()

    nc.compile()
    return nc


def _prep_inputs(inputs):
    """Host-side weight folding/reshaping. Returns (flags, common_map, x)."""
    x = np.ascontiguousarray(np.asarray(inputs["x"], dtype=np.float32))
    g1 = float(np.asarray(inputs["g1"]).reshape(-1)[0])
    be1 = float(np.asarray(inputs["be1"]).reshape(-1)[0])
    g2 = float(np.asarray(inputs["g2"]).reshape(-1)[0])
    be2 = float(np.asarray(inputs["be2"]).reshape(-1)[0])

    wq = np.asarray(inputs["wq"], np.float32)
    wk = np.asarray(inputs["wk"], np.float32)
    wv = np.asarray(inputs["wv"], np.float32)
    wo = np.asarray(inputs["wo"], np.float32)
    w1 = np.asarray(inputs["w1"], np.float32)
    w2 = np.asarray(inputs["w2"], np.float32)

    # LN affine folding (device LN computes (x - mean)/(std + eps) only)
    bq = np.asarray(inputs["bq"], np.float32) + be1 * wq.sum(axis=0)
    bk = np.asarray(inputs["bk"], np.float32) + be1 * wk.sum(axis=0)
    bv = np.asarray(inputs["bv"], np.float32) + be1 * wv.sum(axis=0)
    bo = np.asarray(inputs["bo"], np.float32)
    b1 = np.asarray(inputs["b1"], np.float32) + be2 * w1.sum(axis=0)
    b2 = np.asarray(inputs["b2"], np.float32)

    f8np = ml_dtypes.float8_e4m3

    def slab(w, scale, kchunks):
        # [d_in, m] -> [128, kchunks, m] slab layout with prescale (fp32)
        m = w.shape[1]
        return (w * scale).reshape(kchunks, P, m).transpose(1, 0, 2)

    w1s = slab(g2 * w1, WS, KC)     # [128, 6, F], x16
    w2s = slab(w2, WS, MC)          # [128, 24, D], x16

    common = {
        "wq8": np.ascontiguousarray(
            np.clip(slab(g1 * wq, WS, KC), -240, 240).astype(f8np)),
        "wk8": np.ascontiguousarray(
            np.clip(slab(g1 * wk, WS, KC), -240, 240).astype(f8np)),
        "wv8": np.ascontiguousarray(
            np.clip(slab(g1 * wv, WS, KC), -240, 240).astype(f8np)),
        "wo8": np.ascontiguousarray(
            np.clip(slab(wo, WS, KC), -240, 240).astype(f8np)),
        "w18": np.ascontiguousarray(
            np.clip(w1s[:, :F1_8], -240, 240).astype(f8np)),
        "w1h": np.ascontiguousarray(w1s[:, F1_8:].astype(np.float16)),
        "w28": np.ascontiguousarray(
            np.clip(w2s[:, :F2_8], -240, 240).astype(f8np)),
        "w2h": np.ascontiguousarray(w2s[:, F2_8:].astype(np.float16)),
    }
    flags = (
        bool(np.any(bq) or np.any(bk)),
        bool(np.any(bv)),
        bool(np.any(bo)),
        bool(np.any(b1)),
        bool(np.any(b2)),
    )
    has_bqk, has_bv, has_bo, has_b1, has_b2 = flags
    if has_bqk:
        common["bq"] = np.ascontiguousarray(WS * bq)
        common["bk"] = np.ascontiguousarray(WS * bk)
    if has_bv:
        common["bv"] = np.ascontiguousarray(WS * bv)
    if has_bo:
        common["bo"] = np.ascontiguousarray(bo)
    if has_b1:
        common["b1"] = np.ascontiguousarray(b1)
    if has_b2:
        common["b2"] = np.ascontiguousarray(b2)
    return flags, common, x


def kernel(**inputs):
    global LAST_RESULT
    flags, common, x = _prep_inputs(inputs)
    if flags not in _COMPILE_CACHE:
        _COMPILE_CACHE[flags] = _build(flags)
    nc = _COMPILE_CACHE[flags]

    n_cores = x.shape[0]
    in_maps = [dict(common, x=np.ascontiguousarray(x[i])) for i in range(n_cores)]
    trace = os.environ.get("BASS_KERNEL_TRACE") == "1"
    res = run_bass_kernel_spmd(nc, in_maps, list(range(n_cores)), trace=trace)
    LAST_RESULT = res
    out = np.stack([res.results[i]["out"] for i in range(n_cores)], axis=0)
    return out.astype(np.float32)


# revision 29
# speedup vs baseline: 1.0141x; 1.0141x over previous
"""Trainium2 Bass kernel for an 8x1024x768 pre-LN transformer encoder block.

Sharding: data-parallel over batch - 8 batch elements -> 8 NeuronCores, no
collectives. Each core runs the full block on its [1024, 768] slice.

v4 strategy (baseline v2 = 345us, v3 = 300us but over the error gate):
  - Attention kept in v2 shape: fp8-DoubleRow QKV/O projections, PE
    row-tiled fp8 scores, bf16 softmax/PV (dense PE work keeps the HAM
    clock-gate warm through the ACT-exp-bound attention phase).
  - FFN mixed precision: 1/3 of both FFN contractions run fp8e4 DoubleRow
    (w prescaled x16), the rest fp16 (free accuracy win over bf16).
    Halves a third of the dominant 123us FFN TensorE time while staying
    under the 2e-2 error gate (measured 1.5e-2 in simulation).
  - LN uses a single Rsqrt (folds sqrt+eps+reciprocal; eps contributes
    ~1e-5 relative, negligible).
  - Startup: x chunks stream on the sync+scalar HWDGE queues ahead of the
    attention weights, so the first LN chunks aren't stuck behind weight
    packets on the shared SDMA engines; w1(fp8 part)/wo prefetch during
    attention; outputs spread across three queues.
  - Matmul loops ordered so consecutive matmuls share the stationary
    operand (halves LDWEIGHTS traffic).

Residual stream, LN stats, PSUM accumulation stay fp32.
"""

import os

import numpy as np
import ml_dtypes

import concourse.bass as bass
import concourse.mybir as mybir
import concourse.tile as tile
from concourse import bacc
from concourse.bass_utils import run_bass_kernel_spmd
from concourse.masks import make_identity

P = 128
NT = 1024          # tokens per core
NI = NT // P       # 8 token chunks
D = 768
KC = D // P        # 6 feature chunks
KP = KC // 2       # 3 DoubleRow feature-chunk pairs
H = 12
DH = 64
F = 3072
MC = F // P        # 24 ffn chunks
EPS = 1e-5
WS = 16.0          # fp8 weight prescale
DR = mybir.MatmulPerfMode.DoubleRow

F1_8 = 2           # of KC=6 d_model chunks of FFN1 contraction in fp8-DR
F2_8 = 10          # of MC=24 d_ff chunks of FFN2 contraction in fp8-DR

f32 = mybir.dt.float32
bf16 = mybir.dt.bfloat16
f16 = mybir.dt.float16
f8 = mybir.dt.float8e4

_COMPILE_CACHE = {}
LAST_RESULT = None  # BassKernelResults of the most recent run (for test harness)


def _build(flags):
    has_bqk, has_bv, has_bo, has_b1, has_b2 = flags
    nc = bacc.Bacc("TRN2", target_bir_lowering=False, debug=False, num_devices=8)

    x_d = nc.dram_tensor("x", [NT, D], f32, kind="ExternalInput").ap()
    # fp8 attention weights, pre-reshaped host-side to [128, KC, D]:
    # element [p, kc, m] = scaled_w[kc*128 + p, m]
    wq_d = nc.dram_tensor("wq8", [P, KC, D], f8, kind="ExternalInput").ap()
    wk_d = nc.dram_tensor("wk8", [P, KC, D], f8, kind="ExternalInput").ap()
    wv_d = nc.dram_tensor("wv8", [P, KC, D], f8, kind="ExternalInput").ap()
    wo_d = nc.dram_tensor("wo8", [P, KC, D], f8, kind="ExternalInput").ap()
    # FFN weights, host-prescaled by 16: fp8 slabs for the DR chunks,
    # fp16 slabs for the rest.
    w18_d = nc.dram_tensor("w18", [P, F1_8, F], f8, kind="ExternalInput").ap()
    w1h_d = nc.dram_tensor("w1h", [P, KC - F1_8, F], f16, kind="ExternalInput").ap()
    w28_d = nc.dram_tensor("w28", [P, F2_8, D], f8, kind="ExternalInput").ap()
    w2h_d = nc.dram_tensor("w2h", [P, MC - F2_8, D], f16, kind="ExternalInput").ap()
    bq_d = nc.dram_tensor("bq", [D], f32, kind="ExternalInput").ap() if has_bqk else None
    bk_d = nc.dram_tensor("bk", [D], f32, kind="ExternalInput").ap() if has_bqk else None
    bv_d = nc.dram_tensor("bv", [D], f32, kind="ExternalInput").ap() if has_bv else None
    bo_d = nc.dram_tensor("bo", [D], f32, kind="ExternalInput").ap() if has_bo else None
    b1_d = nc.dram_tensor("b1", [F], f32, kind="ExternalInput").ap() if has_b1 else None
    b2_d = nc.dram_tensor("b2", [D], f32, kind="ExternalInput").ap() if has_b2 else None
    out_d = nc.dram_tensor("out", [NT, D], f32, kind="ExternalOutput").ap()

    with tile.TileContext(nc) as tc:
        sb = tc.alloc_tile_pool(name="sb", bufs=1, space="SBUF")
        ps = tc.alloc_tile_pool(name="ps", bufs=1, space="PSUM")

        # ---- constants ----
        ident = sb.tile([P, P], bf16, tag="ident", bufs=1, name="ident")
        make_identity(nc, ident)
        nb2 = sb.tile([P, 1], f32, tag="nb2", bufs=1, name="nb2")
        nc.vector.memset(nb2, -2.0)
        # 128*I in fp32: lets the O-proj residual ride the PSUM accumulator
        # (psum holds 128*(o + x) after one extra fp32 matmul).
        id128 = sb.tile([P, P], f32, tag="id128", bufs=1, name="id128")
        nc.gpsimd.memset(id128, 0.0)
        nc.gpsimd.affine_select(
            out=id128, in_=id128,
            compare_op=mybir.AluOpType.not_equal, fill=128.0,
            base=0, pattern=[[-1, P]], channel_multiplier=1,
        )

        def bcast_row(src_ap, n, name):
            t = sb.tile([P, n], f32, tag=name, bufs=1, name=name)
            nc.sync.dma_start(
                out=t,
                in_=bass.AP(
                    tensor=src_ap.tensor, offset=src_ap.offset, ap=[[0, P], [1, n]]
                ),
            )
            return t

        def chunk_vec(src_ap, nchunk, name):
            t = sb.tile([P, nchunk], f32, tag=name, bufs=1, name=name)
            nc.sync.dma_start(
                out=t,
                in_=bass.AP(
                    tensor=src_ap.tensor,
                    offset=src_ap.offset,
                    ap=[[1, P], [P, nchunk]],
                ),
            )
            return t

        bq_sb = chunk_vec(bq_d, KC, "bq_sb") if has_bqk else None
        bk_sb = chunk_vec(bk_d, KC, "bk_sb") if has_bqk else None
        b1_sb = chunk_vec(b1_d, MC, "b1_sb") if has_b1 else None
        bv_bc = bcast_row(bv_d, D, "bv_bc") if has_bv else None
        bo_bc = bcast_row(bo_d, D, "bo_bc") if has_bo else None
        b2_bc = bcast_row(b2_d, D, "b2_bc") if has_b2 else None

        # ---- persistent fp8 attention weights ----
        wq8 = sb.tile([P, KC, D], f8, tag="w8", bufs=4, name="wq8")
        wk8 = sb.tile([P, KC, D], f8, tag="w8", bufs=4, name="wk8")
        wv8 = sb.tile([P, KC, D], f8, tag="w8", bufs=4, name="wv8")
        wo8 = sb.tile([P, KC, D], f8, tag="w8", bufs=4, name="wo8")
        # persistent fp8 part of w1 (prefetched during attention); the
        # second "wf" slot later holds the fp8 part of w2.
        w18 = sb.tile([P, F1_8, F], f8, tag="wf", bufs=2, name="w18")

        # ---- persistent activations ----
        # Tag sharing (disjoint lifetimes -> same slots):
        #   bigA: xnT (LN1 -> last qk read)  /  hTh (FFN1 -> FFN2)
        #   bigB: v65 (LN1 -> last PV read)  /  hT8 (FFN1 -> FFN2)
        #   qkx:  qt,kt (-> last scores read) / xn2T8,xn2Th (LN2 -> FFN1)
        x_t = [sb.tile([P, D], f32, tag="x", bufs=NI, name=f"x{i}") for i in range(NI)]
        xnT = sb.tile([P, KC, NT], f8, tag="bigA", bufs=1, name="xnT")
        qt = sb.tile([P, KC, NT], f8, tag="qkx", bufs=2, name="qt")
        kt = sb.tile([P, KC, NT], f8, tag="qkx", bufs=2, name="kt")
        v65 = sb.tile([P, NI, 848], bf16, tag="bigB", bufs=1, name="v65")
        ot = sb.tile([P, KC, NT], f8, tag="ot", bufs=1, name="ot")

        def layernorm(src, dst, i):
            """dst = (src - mean(src)) / (std_unbiased(src) + eps), rowwise.

            Stats on DVE; the normalize itself runs on ACT as an affine
            Identity (scale=rstd, bias=-mean*rstd) so DVE can start the
            next chunk's stats in parallel.
            """
            st = sb.tile([P, 2, 6], f32, tag="stat", bufs=2, name=f"st{i}")
            xg = src.rearrange("p (s f) -> p s f", f=384)
            for s in range(2):
                nc.vector.bn_stats(out=st[:, s, :], in_=xg[:, s, :])
            mv = sb.tile([P, 2], f32, tag="mv", bufs=2, name=f"mv{i}")
            nc.vector.bn_aggr(out=mv, in_=st)
            sd = sb.tile([P, 1], f32, tag="sd", bufs=4, name=f"sd{i}")
            nc.scalar.activation(
                out=sd, in_=mv[:, 1:2], func=mybir.ActivationFunctionType.Sqrt,
                scale=float(D) / float(D - 1),
            )
            nc.vector.tensor_scalar_add(out=sd, in0=sd, scalar1=EPS)
            rstd = sb.tile([P, 1], f32, tag="sd", bufs=4, name=f"rstd{i}")
            nc.vector.reciprocal(out=rstd, in_=sd)
            nmr = sb.tile([P, 1], f32, tag="sd", bufs=4, name=f"nmr{i}")
            nc.vector.tensor_scalar(
                out=nmr, in0=mv[:, 0:1], scalar1=rstd, scalar2=-1.0,
                op0=mybir.AluOpType.mult, op1=mybir.AluOpType.mult,
            )
            nc.scalar.activation(
                out=dst, in_=src, func=mybir.ActivationFunctionType.Identity,
                scale=rstd, bias=nmr,
            )

        def transpose_chunks(xsrc, i, pfx):
            tp = ps.tile([P, D], bf16, tag="stp", bufs=3, name=f"{pfx}{i}")
            for k in range(KC):
                nc.tensor.transpose(
                    tp[:, k * P : (k + 1) * P], xsrc[:, k * P : (k + 1) * P], ident
                )
            return tp

        def v_proj(j):
            vaccs = [
                ps.tile([P, 512], f32, tag="smp", bufs=2, name=f"vps{j}_{hf}")
                for hf in range(2)
            ]
            for kp in range(KP):
                for hf in range(2):
                    nc.tensor.matmul(
                        vaccs[hf][:, 0:384],
                        xnT[:, 2 * kp : 2 * kp + 2, j * P : (j + 1) * P],
                        wv8[:, 2 * kp : 2 * kp + 2, hf * 384 : (hf + 1) * 384],
                        start=(kp == 0), stop=(kp == KP - 1),
                        perf_mode=DR,
                    )
            for hf in range(2):
                acc = vaccs[hf]
                vview = v65[:, j, hf * 390 : hf * 390 + 390].rearrange(
                    "p (h c) -> p h c", c=DH + 1
                )
                dst = vview[:, :, 0:DH]
                src = acc[:, 0:384].rearrange("p (h c) -> p h c", h=6)
                if has_bv:
                    nc.vector.tensor_add(
                        out=dst, in0=src,
                        in1=bv_bc[:, hf * 384 : (hf + 1) * 384].rearrange(
                            "p (h c) -> p h c", h=6
                        ),
                    )
                else:
                    nc.vector.tensor_copy(out=dst, in_=src)


        # ---- per token chunk: load, LN1, transpose, V projection ----
        # x chunks go FIRST on all three DMA-issue queues; the attention-
        # weight DMAs queue behind them (shared SDMA engines drain x first).
        # ones-column value 2.0: v65 holds 16*v, denominator row gets
        # 2*sum(p) -> normalize yields 8*attn (descale folded). Constant,
        # so written up front off the critical path.
        for j in range(NI):
            nc.vector.memset(
                v65[:, j, 0:780].rearrange("p (h c) -> p h c", c=DH + 1)[:, :, DH:],
                2.0,
            )
            nc.vector.memset(v65[:, j, 780:848], 0.0)

        for i in range(NI):
            q = (nc.sync, nc.scalar, nc.gpsimd)[i % 3]
            q.dma_start(out=x_t[i], in_=x_d[i * P : (i + 1) * P, :])
        nc.sync.dma_start(out=wv8, in_=wv_d)
        nc.scalar.dma_start(out=wk8, in_=wk_d)
        nc.scalar.dma_start(out=wq8, in_=wq_d)
        # PE warm-up: ~7us of dummy transposes while the x DMAs land, so
        # the HAM clock-gate is already at 8/8 when the real work starts.
        wrm = ps.tile([P, P], bf16, tag="smp", bufs=2, name="wrm")
        for _ in range(50):
            nc.tensor.transpose(wrm, ident, ident)

        for i in range(NI):
            xn = sb.tile([P, D], bf16, tag="xn", bufs=2, name=f"xn{i}")
            layernorm(x_t[i], xn, i)
            tp = transpose_chunks(xn, i, "tp")
            nc.scalar.copy(
                out=xnT[:, :, i * P : (i + 1) * P],
                in_=tp.rearrange("p (k c) -> p k c", k=KC),
            )
            v_proj(i)

        # ---- QK projections (fp8 DoubleRow) + row-tiled scores + bf16 PV ----
        EXP_SCALE = 0.125 / (WS * WS)

        # wo needed right after attention; w1's fp8 slab prefetched under
        # the attention phase (HBM is otherwise idle there).
        nc.sync.dma_start(out=wo8, in_=wo_d)
        nc.sync.dma_start(out=w18, in_=w18_d)

        def qk_proj(m, w8, b_sb, dstT, nm):
            acc = ps.tile([P, NT], f32, tag="stp", bufs=3, name=f"{nm}ps{m}")
            for kp in range(KP):
                for ih in range(2):
                    nc.tensor.matmul(
                        acc[:, ih * 512 : (ih + 1) * 512],
                        w8[:, 2 * kp : 2 * kp + 2, m * P : (m + 1) * P],
                        xnT[:, 2 * kp : 2 * kp + 2, ih * 512 : (ih + 1) * 512],
                        start=(kp == 0), stop=(kp == KP - 1),
                        perf_mode=DR,
                    )
            for ih in range(2):
                hsl = slice(ih * 512, (ih + 1) * 512)
                if has_bqk:
                    nc.vector.tensor_scalar_add(
                        out=dstT[:, m, hsl], in0=acc[:, hsl],
                        scalar1=b_sb[:, m : m + 1],
                    )
                else:
                    nc.vector.tensor_copy(out=dstT[:, m, hsl], in_=acc[:, hsl])

        def attn_scores_j(m, j, pts):
            # heads 2m (rows 0:64) and 2m+1 (rows 64:128), row-tiled pair.
            stps = [
                ps.tile([P, NT], f32, tag="stp", bufs=3, name=f"st{2 * m + hh}_{j}")
                for hh in range(2)
            ]
            for hh in range(2):
                r0, r1 = hh * DH, (hh + 1) * DH
                for ih in range(2):
                    nc.tensor.matmul(
                        stps[hh][:, ih * 512 : (ih + 1) * 512],
                        kt[r0:r1, m, j * P : (j + 1) * P],
                        qt[r0:r1, m, ih * 512 : (ih + 1) * 512],
                        start=True, stop=True,
                    )
            for hh in range(2):
                ptj = sb.tile(
                    [P, NT], bf16, tag="pt", bufs=16, name=f"pt{2 * m + hh}_{j}"
                )
                nc.scalar.activation(
                    out=ptj, in_=stps[hh],
                    func=mybir.ActivationFunctionType.Exp,
                    scale=EXP_SCALE, bias=nb2,
                )
                pts[hh].append(ptj)

        def attn_pv(h, pt_h):
            p_, hh = divmod(h, 2)
            r0, r1 = hh * DH, (hh + 1) * DH
            opv = [
                ps.tile([P, 512], f32, tag="smp", bufs=2, name=f"opv{h}_{iq}")
                for iq in range(2)
            ]
            for j in range(NI):
                for iq in range(2):
                    nc.tensor.matmul(
                        opv[iq],
                        v65[:, j, h * (DH + 1) : h * (DH + 1) + P],
                        pt_h[j][:, iq * 512 : (iq + 1) * 512],
                        start=(j == 0), stop=(j == NI - 1),
                    )
            for iq in range(2):
                ob = sb.tile([DH + 1, 512], f32, tag="ob", bufs=2, name=f"ob{h}_{iq}")
                nc.vector.tensor_copy(out=ob, in_=opv[iq][0 : DH + 1, :])
                dsb = sb.tile([1, 512], f32, tag="dsb", bufs=2, name=f"dsb{h}_{iq}")
                nc.vector.tensor_copy(out=dsb, in_=ob[DH : DH + 1, :])
                rc = sb.tile([1, 512], f32, tag="rc", bufs=2, name=f"rc{h}_{iq}")
                nc.vector.reciprocal_approx_fast(out=rc, in_=dsb)
                rb = sb.tile([DH, 512], f32, tag="rb", bufs=2, name=f"rb{h}_{iq}")
                nc.gpsimd.partition_broadcast(rb, rc)
                nc.vector.tensor_mul(
                    out=ot[r0:r1, p_, iq * 512 : (iq + 1) * 512],
                    in0=ob[0:DH, :], in1=rb,
                )

        # Software-pipelined attention: scores/exp stream continuously on
        # PE/ACT; the previous head-pair's PV and the next chunk's QK
        # projections are emitted into the gaps so the exp stream (the
        # phase's critical resource) never starves.
        qk_proj(0, wk8, bk_sb, kt, "k")
        qk_proj(0, wq8, bq_sb, qt, "q")
        prev = None
        for m in range(KC):
            pts = [[], []]
            for j in range(NI):
                attn_scores_j(m, j, pts)
                if j == 1 and prev is not None:
                    attn_pv(2 * prev[0], prev[1][0])
                if j == 3 and prev is not None:
                    attn_pv(2 * prev[0] + 1, prev[1][1])
                if j == 5 and m + 1 < KC:
                    qk_proj(m + 1, wk8, bk_sb, kt, "k")
                if j == 7 and m + 1 < KC:
                    qk_proj(m + 1, wq8, bq_sb, qt, "q")
            prev = (m, pts)
        attn_pv(2 * prev[0], prev[1][0])
        attn_pv(2 * prev[0] + 1, prev[1][1])

        # ---- O-proj (fp8 DoubleRow, psum = 128*o), residual, LN2, transpose ----
        xn2T8 = sb.tile([P, F1_8, NT], f8, tag="qkx", bufs=2, name="xn2T8")
        xn2Th = sb.tile([P, KC - F1_8, NT], f16, tag="qkx", bufs=2, name="xn2Th")
        for i in range(NI):
            oaccs = [
                ps.tile([P, 512], f32, tag="smp", bufs=2, name=f"ops{i}_{hf}")
                for hf in range(2)
            ]
            for cp in range(KP):
                for hf in range(2):
                    nc.tensor.matmul(
                        oaccs[hf][:, 0:384],
                        ot[:, 2 * cp : 2 * cp + 2, i * P : (i + 1) * P],
                        wo8[:, 2 * cp : 2 * cp + 2, hf * 384 : (hf + 1) * 384],
                        start=(cp == 0), stop=(cp == KP - 1),
                        perf_mode=DR,
                    )
            for hf in range(2):
                xsl = x_t[i][:, hf * 384 : (hf + 1) * 384]
                nc.vector.scalar_tensor_tensor(
                    out=xsl, in0=oaccs[hf][:, 0:384], scalar=1.0 / (8.0 * WS),
                    in1=xsl,
                    op0=mybir.AluOpType.mult, op1=mybir.AluOpType.add,
                )
                if has_bo:
                    nc.vector.tensor_add(
                        out=xsl, in0=xsl, in1=bo_bc[:, hf * 384 : (hf + 1) * 384]
                    )
            xn2 = sb.tile([P, D], bf16, tag="xn", bufs=2, name=f"xn2_{i}")
            layernorm(x_t[i], xn2, NI + i)
            tq = transpose_chunks(xn2, i, "tq")
            nc.scalar.copy(
                out=xn2T8[:, :, i * P : (i + 1) * P],
                in_=tq[:, 0 : F1_8 * P].rearrange("p (k c) -> p k c", k=F1_8),
            )
            nc.scalar.copy(
                out=xn2Th[:, :, i * P : (i + 1) * P],
                in_=tq[:, F1_8 * P : D].rearrange("p (k c) -> p k c", k=KC - F1_8),
            )

        # ---- FFN1 mixed fp8-DR + fp16: acc = 16*(xn2 @ w1); gelu(acc/16) ----
        hT8 = sb.tile([P, F2_8, NT], f8, tag="bigB", bufs=1, name="hT8")
        hTh = sb.tile([P, MC - F2_8, NT], f16, tag="bigA", bufs=1, name="hTh")
        # persist w2 for FFN2: fp8 slab in the second "wf" slot; fp16 chunks
        # parked in the (now dead) softmax "pt" slots. FFN2 then has no DMA
        # dependence at all.
        w28p = sb.tile([P, F2_8, D], f8, tag="wf", bufs=2, name="w28p")
        nc.sync.dma_start(out=w28p, in_=w28_d)
        w2ht = [
            sb.tile([P, D], f16, tag="pt", bufs=16, name=f"w2h{k}")
            for k in range(MC - F2_8)
        ]
        for mp in range(MC // 2):
            w1hs = sb.tile([P, KC - F1_8, 2 * P], f16, tag="w1hs", bufs=3,
                           name=f"w1hs{mp}")
            nc.sync.dma_start(
                out=w1hs, in_=w1h_d[:, :, mp * 2 * P : (mp + 1) * 2 * P]
            )
            if mp >= 4:
                for t2 in range(2):
                    k = 2 * (mp - 4) + t2
                    if k < MC - F2_8:
                        nc.sync.dma_start(out=w2ht[k], in_=w2h_d[:, k, :])
            for t in range(2):
                m = 2 * mp + t
                acc = ps.tile([P, NT], f32, tag="stp", bufs=3, name=f"fps{m}")
                for ih in range(2):
                    nc.tensor.matmul(
                        acc[:, ih * 512 : (ih + 1) * 512],
                        w18[:, 0:F1_8, m * P : (m + 1) * P],
                        xn2T8[:, 0:F1_8, ih * 512 : (ih + 1) * 512],
                        start=True, stop=False,
                        perf_mode=DR,
                    )
                for c in range(KC - F1_8):
                    for ih in range(2):
                        nc.tensor.matmul(
                            acc[:, ih * 512 : (ih + 1) * 512],
                            w1hs[:, c, t * P : (t + 1) * P],
                            xn2Th[:, c, ih * 512 : (ih + 1) * 512],
                            start=False, stop=(c == KC - F1_8 - 1),
                        )
                hdst = hT8[:, m, :] if m < F2_8 else hTh[:, m - F2_8, :]
                nc.scalar.activation(
                    out=hdst, in_=acc,
                    func=mybir.ActivationFunctionType.Gelu,
                    scale=1.0 / WS,
                    bias=b1_sb[:, m : m + 1] if has_b1 else 0.0,
                )

        # ---- FFN2 mixed fp8-DR + fp16: per token half, 8 live accumulators ----
        for ihp in range(2):
            stps = [
                ps.tile([P, NT], f32, tag="stp", bufs=3, name=f"f2s{ihp}_{sl}")
                for sl in range(3)
            ]
            smps = [
                ps.tile([P, 512], f32, tag="smp", bufs=2, name=f"f2m{ihp}_{sl}")
                for sl in range(2)
            ]
            def accsl(il, dh):
                q = 2 * il + dh
                if q < 6:
                    return stps[q // 2][:, (q % 2) * 512 : (q % 2) * 512 + 384]
                return smps[q - 6][:, 0:384]
            # il-outer: ~40 consecutive matmuls per PSUM bank pair instead
            # of cycling all 8 banks every 8 matmuls (avoids the psum-queue
            # HAM oscillation), and lets each token chunk's residual start
            # while the next accumulates.
            NKH = MC - F2_8
            for il in range(4):
                tok = slice((4 * ihp + il) * P, (4 * ihp + il + 1) * P)
                for c in range(F2_8 // 2):
                    for dh_ in range(2):
                        nc.tensor.matmul(
                            accsl(il, dh_),
                            hT8[:, 2 * c : 2 * c + 2, tok],
                            w28p[:, 2 * c : 2 * c + 2, dh_ * 384 : (dh_ + 1) * 384],
                            start=(c == 0), stop=False,
                            perf_mode=DR,
                        )
                for kk in range(NKH):
                    for dh_ in range(2):
                        nc.tensor.matmul(
                            accsl(il, dh_),
                            hTh[:, kk, tok],
                            w2ht[kk][:, dh_ * 384 : (dh_ + 1) * 384],
                            start=False, stop=(kk == NKH - 1),
                        )
            for il in range(4):
                i = 4 * ihp + il
                for dh_ in range(2):
                    xsl = x_t[i][:, dh_ * 384 : (dh_ + 1) * 384]
                    nc.vector.scalar_tensor_tensor(
                        out=xsl, in0=accsl(il, dh_), scalar=1.0 / WS,
                        in1=xsl,
                        op0=mybir.AluOpType.mult, op1=mybir.AluOpType.add,
                    )
                    if has_b2:
                        nc.vector.tensor_add(
                            out=xsl, in0=xsl, in1=b2_bc[:, dh_ * 384 : (dh_ + 1) * 384]
                        )
                q = (nc.gpsimd, nc.sync, nc.scalar, nc.gpsimd)[il]
                q.dma_start(out=out_d[i * P : (i + 1) * P, :], in_=x_t[i])

        sb.release()
        ps.release()

    nc.compile()
    return nc


def _prep_inputs(inputs):
    """Host-side weight folding/reshaping. Returns (flags, common_map, x)."""
    x = np.ascontiguousarray(np.asarray(inputs["x"], dtype=np.float32))
    g1 = float(np.asarray(inputs["g1"]).reshape(-1)[0])
    be1 = float(np.asarray(inputs["be1"]).reshape(-1)[0])
    g2 = float(np.asarray(inputs["g2"]).reshape(-1)[0])
    be2 = float(np.asarray(inputs["be2"]).reshape(-1)[0])

    wq = np.asarray(inputs["wq"], np.float32)
    wk = np.asarray(inputs["wk"], np.float32)
    wv = np.asarray(inputs["wv"], np.float32)
    wo = np.asarray(inputs["wo"], np.float32)
    w1 = np.asarray(inputs["w1"], np.float32)
    w2 = np.asarray(inputs["w2"], np.float32)

    # LN affine folding (device LN computes (x - mean)/(std + eps) only)
    bq = np.asarray(inputs["bq"], np.float32) + be1 * wq.sum(axis=0)
    bk = np.asarray(inputs["bk"], np.float32) + be1 * wk.sum(axis=0)
    bv = np.asarray(inputs["bv"], np.float32) + be1 * wv.sum(axis=0)
    bo = np.asarray(inputs["bo"], np.float32)
    b1 = np.asarray(inputs["b1"], np.float32) + be2 * w1.sum(axis=0)
    b2 = np.asarray(inputs["b2"], np.float32)

    f8np = ml_dtypes.float8_e4m3

    def slab(w, scale, kchunks):
        # [d_in, m] -> [128, kchunks, m] slab layout with prescale (fp32)
        m = w.shape[1]
        return (w * scale).reshape(kchunks, P, m).transpose(1, 0, 2)

    w1s = slab(g2 * w1, WS, KC)     # [128, 6, F], x16
    w2s = slab(w2, WS, MC)          # [128, 24, D], x16

    common = {
        "wq8": np.ascontiguousarray(
            np.clip(slab(g1 * wq, WS, KC), -240, 240).astype(f8np)),
        "wk8": np.ascontiguousarray(
            np.clip(slab(g1 * wk, WS, KC), -240, 240).astype(f8np)),
        "wv8": np.ascontiguousarray(
            np.clip(slab(g1 * wv, WS, KC), -240, 240).astype(f8np)),
        "wo8": np.ascontiguousarray(
            np.clip(slab(wo, WS, KC), -240, 240).astype(f8np)),
        "w18": np.ascontiguousarray(
            np.clip(w1s[:, :F1_8], -240, 240).astype(f8np)),
        "w1h": np.ascontiguousarray(w1s[:, F1_8:].astype(np.float16)),
        "w28": np.ascontiguousarray(
            np.clip(w2s[:, :F2_8], -240, 240).astype(f8np)),
        "w2h": np.ascontiguousarray(w2s[:, F2_8:].astype(np.float16)),
    }
    flags = (
        bool(np.any(bq) or np.any(bk)),
        bool(np.any(bv)),
        bool(np.any(bo)),
        bool(np.any(b1)),
        bool(np.any(b2)),
    )
    has_bqk, has_bv, has_bo, has_b1, has_b2 = flags
    if has_bqk:
        common["bq"] = np.ascontiguousarray(WS * bq)
        common["bk"] = np.ascontiguousarray(WS * bk)
    if has_bv:
        common["bv"] = np.ascontiguousarray(WS * bv)
    if has_bo:
        common["bo"] = np.ascontiguousarray(bo)
    if has_b1:
        common["b1"] = np.ascontiguousarray(b1)
    if has_b2:
        common["b2"] = np.ascontiguousarray(b2)
    return flags, common, x


def kernel(**inputs):
    global LAST_RESULT
    flags, common, x = _prep_inputs(inputs)
    if flags not in _COMPILE_CACHE:
        _COMPILE_CACHE[flags] = _build(flags)
    nc = _COMPILE_CACHE[flags]

    n_cores = x.shape[0]
    in_maps = [dict(common, x=np.ascontiguousarray(x[i])) for i in range(n_cores)]
    trace = os.environ.get("BASS_KERNEL_TRACE") == "1"
    res = run_bass_kernel_spmd(nc, in_maps, list(range(n_cores)), trace=trace)
    LAST_RESULT = res
    out = np.stack([res.results[i]["out"] for i in range(n_cores)], axis=0)
    return out.astype(np.float32)


# revision 30
# speedup vs baseline: 1.0166x; 1.0025x over previous
"""Trainium2 Bass kernel for an 8x1024x768 pre-LN transformer encoder block.

Sharding: data-parallel over batch - 8 batch elements -> 8 NeuronCores, no
collectives. Each core runs the full block on its [1024, 768] slice.

v4 strategy (baseline v2 = 345us, v3 = 300us but over the error gate):
  - Attention kept in v2 shape: fp8-DoubleRow QKV/O projections, PE
    row-tiled fp8 scores, bf16 softmax/PV (dense PE work keeps the HAM
    clock-gate warm through the ACT-exp-bound attention phase).
  - FFN mixed precision: 1/3 of both FFN contractions run fp8e4 DoubleRow
    (w prescaled x16), the rest fp16 (free accuracy win over bf16).
    Halves a third of the dominant 123us FFN TensorE time while staying
    under the 2e-2 error gate (measured 1.5e-2 in simulation).
  - LN uses a single Rsqrt (folds sqrt+eps+reciprocal; eps contributes
    ~1e-5 relative, negligible).
  - Startup: x chunks stream on the sync+scalar HWDGE queues ahead of the
    attention weights, so the first LN chunks aren't stuck behind weight
    packets on the shared SDMA engines; w1(fp8 part)/wo prefetch during
    attention; outputs spread across three queues.
  - Matmul loops ordered so consecutive matmuls share the stationary
    operand (halves LDWEIGHTS traffic).

Residual stream, LN stats, PSUM accumulation stay fp32.
"""

import os

import numpy as np
import ml_dtypes

import concourse.bass as bass
import concourse.mybir as mybir
import concourse.tile as tile
from concourse import bacc
from concourse.bass_utils import run_bass_kernel_spmd
from concourse.masks import make_identity

P = 128
NT = 1024          # tokens per core
NI = NT // P       # 8 token chunks
D = 768
KC = D // P        # 6 feature chunks
KP = KC // 2       # 3 DoubleRow feature-chunk pairs
H = 12
DH = 64
F = 3072
MC = F // P        # 24 ffn chunks
EPS = 1e-5
WS = 16.0          # fp8 weight prescale
DR = mybir.MatmulPerfMode.DoubleRow

F1_8 = 2           # of KC=6 d_model chunks of FFN1 contraction in fp8-DR
F2_8 = 8           # of MC=24 d_ff chunks of FFN2 contraction in fp8-DR

f32 = mybir.dt.float32
bf16 = mybir.dt.bfloat16
f16 = mybir.dt.float16
f8 = mybir.dt.float8e4

_COMPILE_CACHE = {}
LAST_RESULT = None  # BassKernelResults of the most recent run (for test harness)


def _build(flags):
    has_bqk, has_bv, has_bo, has_b1, has_b2 = flags
    nc = bacc.Bacc("TRN2", target_bir_lowering=False, debug=False, num_devices=8)

    x_d = nc.dram_tensor("x", [NT, D], f32, kind="ExternalInput").ap()
    # fp8 attention weights, pre-reshaped host-side to [128, KC, D]:
    # element [p, kc, m] = scaled_w[kc*128 + p, m]
    wq_d = nc.dram_tensor("wq8", [P, KC, D], f8, kind="ExternalInput").ap()
    wk_d = nc.dram_tensor("wk8", [P, KC, D], f8, kind="ExternalInput").ap()
    wv_d = nc.dram_tensor("wv8", [P, KC, D], f8, kind="ExternalInput").ap()
    wo_d = nc.dram_tensor("wo8", [P, KC, D], f8, kind="ExternalInput").ap()
    # FFN weights, host-prescaled by 16: fp8 slabs for the DR chunks,
    # fp16 slabs for the rest.
    w18_d = nc.dram_tensor("w18", [P, F1_8, F], f8, kind="ExternalInput").ap()
    w1h_d = nc.dram_tensor("w1h", [P, KC - F1_8, F], f16, kind="ExternalInput").ap()
    w28_d = nc.dram_tensor("w28", [P, F2_8, D], f8, kind="ExternalInput").ap()
    w2h_d = nc.dram_tensor("w2h", [P, MC - F2_8, D], f16, kind="ExternalInput").ap()
    bq_d = nc.dram_tensor("bq", [D], f32, kind="ExternalInput").ap() if has_bqk else None
    bk_d = nc.dram_tensor("bk", [D], f32, kind="ExternalInput").ap() if has_bqk else None
    bv_d = nc.dram_tensor("bv", [D], f32, kind="ExternalInput").ap() if has_bv else None
    bo_d = nc.dram_tensor("bo", [D], f32, kind="ExternalInput").ap() if has_bo else None
    b1_d = nc.dram_tensor("b1", [F], f32, kind="ExternalInput").ap() if has_b1 else None
    b2_d = nc.dram_tensor("b2", [D], f32, kind="ExternalInput").ap() if has_b2 else None
    out_d = nc.dram_tensor("out", [NT, D], f32, kind="ExternalOutput").ap()

    with tile.TileContext(nc) as tc:
        sb = tc.alloc_tile_pool(name="sb", bufs=1, space="SBUF")
        ps = tc.alloc_tile_pool(name="ps", bufs=1, space="PSUM")

        # ---- constants ----
        ident = sb.tile([P, P], bf16, tag="ident", bufs=1, name="ident")
        make_identity(nc, ident)
        nb2 = sb.tile([P, 1], f32, tag="nb2", bufs=1, name="nb2")
        nc.vector.memset(nb2, -2.0)

        def bcast_row(src_ap, n, name):
            t = sb.tile([P, n], f32, tag=name, bufs=1, name=name)
            nc.sync.dma_start(
                out=t,
                in_=bass.AP(
                    tensor=src_ap.tensor, offset=src_ap.offset, ap=[[0, P], [1, n]]
                ),
            )
            return t

        def chunk_vec(src_ap, nchunk, name):
            t = sb.tile([P, nchunk], f32, tag=name, bufs=1, name=name)
            nc.sync.dma_start(
                out=t,
                in_=bass.AP(
                    tensor=src_ap.tensor,
                    offset=src_ap.offset,
                    ap=[[1, P], [P, nchunk]],
                ),
            )
            return t

        bq_sb = chunk_vec(bq_d, KC, "bq_sb") if has_bqk else None
        bk_sb = chunk_vec(bk_d, KC, "bk_sb") if has_bqk else None
        b1_sb = chunk_vec(b1_d, MC, "b1_sb") if has_b1 else None
        bv_bc = bcast_row(bv_d, D, "bv_bc") if has_bv else None
        bo_bc = bcast_row(bo_d, D, "bo_bc") if has_bo else None
        b2_bc = bcast_row(b2_d, D, "b2_bc") if has_b2 else None

        # ---- persistent fp8 attention weights ----
        wq8 = sb.tile([P, KC, D], f8, tag="w8", bufs=4, name="wq8")
        wk8 = sb.tile([P, KC, D], f8, tag="w8", bufs=4, name="wk8")
        wv8 = sb.tile([P, KC, D], f8, tag="w8", bufs=4, name="wv8")
        wo8 = sb.tile([P, KC, D], f8, tag="w8", bufs=4, name="wo8")
        # persistent fp8 part of w1 (prefetched during attention); the
        # second "wf" slot later holds the fp8 part of w2.
        w18 = sb.tile([P, F1_8, F], f8, tag="wf", bufs=2, name="w18")

        # ---- persistent activations ----
        # Tag sharing (disjoint lifetimes -> same slots):
        #   bigA: xnT (LN1 -> last qk read)  /  hTh (FFN1 -> FFN2)
        #   bigB: v65 (LN1 -> last PV read)  /  hT8 (FFN1 -> FFN2)
        #   qkx:  qt,kt (-> last scores read) / xn2T8,xn2Th (LN2 -> FFN1)
        x_t = [sb.tile([P, D], f32, tag="x", bufs=NI, name=f"x{i}") for i in range(NI)]
        xnT = sb.tile([P, KC, NT], f8, tag="bigA", bufs=1, name="xnT")
        qt = sb.tile([P, KC, NT], f8, tag="qkx", bufs=2, name="qt")
        kt = sb.tile([P, KC, NT], f8, tag="qkx", bufs=2, name="kt")
        v65 = sb.tile([P, NI, 848], bf16, tag="bigB", bufs=1, name="v65")
        ot = sb.tile([P, KC, NT], f8, tag="ot", bufs=1, name="ot")

        def layernorm(src, dst, i):
            """dst = (src - mean(src)) / (std_unbiased(src) + eps), rowwise.

            Stats on DVE; the normalize itself runs on ACT as an affine
            Identity (scale=rstd, bias=-mean*rstd) so DVE can start the
            next chunk's stats in parallel.
            """
            st = sb.tile([P, 2, 6], f32, tag="stat", bufs=2, name=f"st{i}")
            xg = src.rearrange("p (s f) -> p s f", f=384)
            for s in range(2):
                nc.vector.bn_stats(out=st[:, s, :], in_=xg[:, s, :])
            mv = sb.tile([P, 2], f32, tag="mv", bufs=2, name=f"mv{i}")
            nc.vector.bn_aggr(out=mv, in_=st)
            sd = sb.tile([P, 1], f32, tag="sd", bufs=4, name=f"sd{i}")
            nc.scalar.activation(
                out=sd, in_=mv[:, 1:2], func=mybir.ActivationFunctionType.Sqrt,
                scale=float(D) / float(D - 1),
            )
            nc.vector.tensor_scalar_add(out=sd, in0=sd, scalar1=EPS)
            rstd = sb.tile([P, 1], f32, tag="sd", bufs=4, name=f"rstd{i}")
            nc.vector.reciprocal(out=rstd, in_=sd)
            nmr = sb.tile([P, 1], f32, tag="sd", bufs=4, name=f"nmr{i}")
            nc.vector.tensor_scalar(
                out=nmr, in0=mv[:, 0:1], scalar1=rstd, scalar2=-1.0,
                op0=mybir.AluOpType.mult, op1=mybir.AluOpType.mult,
            )
            nc.scalar.activation(
                out=dst, in_=src, func=mybir.ActivationFunctionType.Identity,
                scale=rstd, bias=nmr,
            )

        def transpose_chunks(xsrc, i, pfx):
            tp = ps.tile([P, D], bf16, tag="stp", bufs=3, name=f"{pfx}{i}")
            for k in range(KC):
                nc.tensor.transpose(
                    tp[:, k * P : (k + 1) * P], xsrc[:, k * P : (k + 1) * P], ident
                )
            return tp

        def v_proj(j):
            vaccs = [
                ps.tile([P, 512], f32, tag="smp", bufs=2, name=f"vps{j}_{hf}")
                for hf in range(2)
            ]
            for kp in range(KP):
                for hf in range(2):
                    nc.tensor.matmul(
                        vaccs[hf][:, 0:384],
                        xnT[:, 2 * kp : 2 * kp + 2, j * P : (j + 1) * P],
                        wv8[:, 2 * kp : 2 * kp + 2, hf * 384 : (hf + 1) * 384],
                        start=(kp == 0), stop=(kp == KP - 1),
                        perf_mode=DR,
                    )
            for hf in range(2):
                acc = vaccs[hf]
                vview = v65[:, j, hf * 390 : hf * 390 + 390].rearrange(
                    "p (h c) -> p h c", c=DH + 1
                )
                dst = vview[:, :, 0:DH]
                src = acc[:, 0:384].rearrange("p (h c) -> p h c", h=6)
                if has_bv:
                    nc.vector.tensor_add(
                        out=dst, in0=src,
                        in1=bv_bc[:, hf * 384 : (hf + 1) * 384].rearrange(
                            "p (h c) -> p h c", h=6
                        ),
                    )
                else:
                    nc.vector.tensor_copy(out=dst, in_=src)
            # ones-column value 2.0: v65 holds 16*v, denominator row gets
            # 2*sum(p) -> normalize yields 8*attn (descale folded).
            nc.vector.memset(
                v65[:, j, 0:780].rearrange("p (h c) -> p h c", c=DH + 1)[:, :, DH:],
                2.0,
            )
            nc.vector.memset(v65[:, j, 780:848], 0.0)

        # ---- per token chunk: load, LN1, transpose, V projection ----
        # x chunks go FIRST on all three DMA-issue queues; the attention-
        # weight DMAs queue behind them (shared SDMA engines drain x first).
        for i in range(NI):
            q = (nc.sync, nc.scalar, nc.gpsimd)[i % 3]
            q.dma_start(out=x_t[i], in_=x_d[i * P : (i + 1) * P, :])
        nc.sync.dma_start(out=wv8, in_=wv_d)
        nc.scalar.dma_start(out=wk8, in_=wk_d)
        nc.scalar.dma_start(out=wq8, in_=wq_d)
        # PE warm-up: ~7us of dummy transposes while the x DMAs land, so
        # the HAM clock-gate is already at 8/8 when the real work starts.
        wrm = ps.tile([P, P], bf16, tag="smp", bufs=2, name="wrm")
        for _ in range(50):
            nc.tensor.transpose(wrm, ident, ident)

        for i in range(NI):
            xn = sb.tile([P, D], bf16, tag="xn", bufs=2, name=f"xn{i}")
            layernorm(x_t[i], xn, i)
            tp = transpose_chunks(xn, i, "tp")
            nc.scalar.copy(
                out=xnT[:, :, i * P : (i + 1) * P],
                in_=tp.rearrange("p (k c) -> p k c", k=KC),
            )
            v_proj(i)

        # ---- QK projections (fp8 DoubleRow) + row-tiled scores + bf16 PV ----
        EXP_SCALE = 0.125 / (WS * WS)

        # wo needed right after attention; w1's fp8 slab prefetched under
        # the attention phase (HBM is otherwise idle there).
        nc.sync.dma_start(out=wo8, in_=wo_d)
        nc.sync.dma_start(out=w18, in_=w18_d)

        def qk_proj(m, w8, b_sb, dstT, nm):
            acc = ps.tile([P, NT], f32, tag="stp", bufs=3, name=f"{nm}ps{m}")
            for kp in range(KP):
                for ih in range(2):
                    nc.tensor.matmul(
                        acc[:, ih * 512 : (ih + 1) * 512],
                        w8[:, 2 * kp : 2 * kp + 2, m * P : (m + 1) * P],
                        xnT[:, 2 * kp : 2 * kp + 2, ih * 512 : (ih + 1) * 512],
                        start=(kp == 0), stop=(kp == KP - 1),
                        perf_mode=DR,
                    )
            for ih in range(2):
                hsl = slice(ih * 512, (ih + 1) * 512)
                if has_bqk:
                    nc.vector.tensor_scalar_add(
                        out=dstT[:, m, hsl], in0=acc[:, hsl],
                        scalar1=b_sb[:, m : m + 1],
                    )
                else:
                    nc.vector.tensor_copy(out=dstT[:, m, hsl], in_=acc[:, hsl])

        def attn_scores_j(m, j, pts):
            # heads 2m (rows 0:64) and 2m+1 (rows 64:128), row-tiled pair.
            stps = [
                ps.tile([P, NT], f32, tag="stp", bufs=3, name=f"st{2 * m + hh}_{j}")
                for hh in range(2)
            ]
            for hh in range(2):
                r0, r1 = hh * DH, (hh + 1) * DH
                for ih in range(2):
                    nc.tensor.matmul(
                        stps[hh][:, ih * 512 : (ih + 1) * 512],
                        kt[r0:r1, m, j * P : (j + 1) * P],
                        qt[r0:r1, m, ih * 512 : (ih + 1) * 512],
                        start=True, stop=True,
                    )
            for hh in range(2):
                ptj = sb.tile(
                    [P, NT], bf16, tag="pt", bufs=20, name=f"pt{2 * m + hh}_{j}"
                )
                nc.scalar.activation(
                    out=ptj, in_=stps[hh],
                    func=mybir.ActivationFunctionType.Exp,
                    scale=EXP_SCALE, bias=nb2,
                )
                pts[hh].append(ptj)

        def attn_pv(h, pt_h):
            p_, hh = divmod(h, 2)
            r0, r1 = hh * DH, (hh + 1) * DH
            opv = [
                ps.tile([P, 512], f32, tag="smp", bufs=2, name=f"opv{h}_{iq}")
                for iq in range(2)
            ]
            for j in range(NI):
                for iq in range(2):
                    nc.tensor.matmul(
                        opv[iq],
                        v65[:, j, h * (DH + 1) : h * (DH + 1) + P],
                        pt_h[j][:, iq * 512 : (iq + 1) * 512],
                        start=(j == 0), stop=(j == NI - 1),
                    )
            for iq in range(2):
                ob = sb.tile([DH + 1, 512], f32, tag="ob", bufs=3, name=f"ob{h}_{iq}")
                nc.vector.tensor_copy(out=ob, in_=opv[iq][0 : DH + 1, :])
                dsb = sb.tile([1, 512], f32, tag="dsb", bufs=3, name=f"dsb{h}_{iq}")
                nc.vector.tensor_copy(out=dsb, in_=ob[DH : DH + 1, :])
                rc = sb.tile([1, 512], f32, tag="rc", bufs=3, name=f"rc{h}_{iq}")
                nc.vector.reciprocal_approx_fast(out=rc, in_=dsb)
                rb = sb.tile([DH, 512], f32, tag="rb", bufs=3, name=f"rb{h}_{iq}")
                nc.gpsimd.partition_broadcast(rb, rc)
                nc.vector.tensor_mul(
                    out=ot[r0:r1, p_, iq * 512 : (iq + 1) * 512],
                    in0=ob[0:DH, :], in1=rb,
                )

        # Software-pipelined attention: scores/exp stream continuously on
        # PE/ACT; the previous head-pair's PV and the next chunk's QK
        # projections are emitted into the gaps so the exp stream (the
        # phase's critical resource) never starves.
        qk_proj(0, wk8, bk_sb, kt, "k")
        qk_proj(0, wq8, bq_sb, qt, "q")
        prev = None
        for m in range(KC):
            pts = [[], []]
            for j in range(NI):
                attn_scores_j(m, j, pts)
                if j == 1 and prev is not None:
                    attn_pv(2 * prev[0], prev[1][0])
                if j == 3 and prev is not None:
                    attn_pv(2 * prev[0] + 1, prev[1][1])
                if j == 5 and m + 1 < KC:
                    qk_proj(m + 1, wk8, bk_sb, kt, "k")
                if j == 7 and m + 1 < KC:
                    qk_proj(m + 1, wq8, bq_sb, qt, "q")
            prev = (m, pts)
        attn_pv(2 * prev[0], prev[1][0])
        attn_pv(2 * prev[0] + 1, prev[1][1])

        # ---- O-proj (fp8 DoubleRow, psum = 128*o), residual, LN2, transpose ----
        xn2T8 = sb.tile([P, F1_8, NT], f8, tag="qkx", bufs=2, name="xn2T8")
        xn2Th = sb.tile([P, KC - F1_8, NT], f16, tag="qkx", bufs=2, name="xn2Th")
        for i in range(NI):
            oaccs = [
                ps.tile([P, 512], f32, tag="smp", bufs=2, name=f"ops{i}_{hf}")
                for hf in range(2)
            ]
            for cp in range(KP):
                for hf in range(2):
                    nc.tensor.matmul(
                        oaccs[hf][:, 0:384],
                        ot[:, 2 * cp : 2 * cp + 2, i * P : (i + 1) * P],
                        wo8[:, 2 * cp : 2 * cp + 2, hf * 384 : (hf + 1) * 384],
                        start=(cp == 0), stop=(cp == KP - 1),
                        perf_mode=DR,
                    )
            for hf in range(2):
                xsl = x_t[i][:, hf * 384 : (hf + 1) * 384]
                nc.vector.scalar_tensor_tensor(
                    out=xsl, in0=oaccs[hf][:, 0:384], scalar=1.0 / (8.0 * WS),
                    in1=xsl,
                    op0=mybir.AluOpType.mult, op1=mybir.AluOpType.add,
                )
                if has_bo:
                    nc.vector.tensor_add(
                        out=xsl, in0=xsl, in1=bo_bc[:, hf * 384 : (hf + 1) * 384]
                    )
            xn2 = sb.tile([P, D], bf16, tag="xn", bufs=2, name=f"xn2_{i}")
            layernorm(x_t[i], xn2, NI + i)
            tq = transpose_chunks(xn2, i, "tq")
            nc.scalar.copy(
                out=xn2T8[:, :, i * P : (i + 1) * P],
                in_=tq[:, 0 : F1_8 * P].rearrange("p (k c) -> p k c", k=F1_8),
            )
            nc.scalar.copy(
                out=xn2Th[:, :, i * P : (i + 1) * P],
                in_=tq[:, F1_8 * P : D].rearrange("p (k c) -> p k c", k=KC - F1_8),
            )

        # ---- FFN1 mixed fp8-DR + fp16: acc = 16*(xn2 @ w1); gelu(acc/16) ----
        hT8 = sb.tile([P, F2_8, NT], f8, tag="bigB", bufs=1, name="hT8")
        hTh = sb.tile([P, MC - F2_8, NT], f16, tag="bigA", bufs=1, name="hTh")
        # persist w2 for FFN2: fp8 slab in the second "wf" slot; fp16 chunks
        # parked in the (now dead) softmax "pt" slots. FFN2 then has no DMA
        # dependence at all.
        w28p = sb.tile([P, F2_8, D], f8, tag="wf", bufs=2, name="w28p")
        nc.sync.dma_start(out=w28p, in_=w28_d)
        w2ht = [
            sb.tile([P, D], f16, tag="pt", bufs=20, name=f"w2h{k}")
            for k in range(MC - F2_8)
        ]
        for mp in range(MC // 2):
            w1hs = sb.tile([P, KC - F1_8, 2 * P], f16, tag="w1hs", bufs=4,
                           name=f"w1hs{mp}")
            nc.sync.dma_start(
                out=w1hs, in_=w1h_d[:, :, mp * 2 * P : (mp + 1) * 2 * P]
            )
            if mp >= 4:
                for t2 in range(2):
                    k = 2 * (mp - 4) + t2
                    nc.sync.dma_start(out=w2ht[k], in_=w2h_d[:, k, :])
            for t in range(2):
                m = 2 * mp + t
                acc = ps.tile([P, NT], f32, tag="stp", bufs=3, name=f"fps{m}")
                for ih in range(2):
                    nc.tensor.matmul(
                        acc[:, ih * 512 : (ih + 1) * 512],
                        w18[:, 0:F1_8, m * P : (m + 1) * P],
                        xn2T8[:, 0:F1_8, ih * 512 : (ih + 1) * 512],
                        start=True, stop=False,
                        perf_mode=DR,
                    )
                for c in range(KC - F1_8):
                    for ih in range(2):
                        nc.tensor.matmul(
                            acc[:, ih * 512 : (ih + 1) * 512],
                            w1hs[:, c, t * P : (t + 1) * P],
                            xn2Th[:, c, ih * 512 : (ih + 1) * 512],
                            start=False, stop=(c == KC - F1_8 - 1),
                        )
                hdst = hT8[:, m, :] if m < F2_8 else hTh[:, m - F2_8, :]
                nc.scalar.activation(
                    out=hdst, in_=acc,
                    func=mybir.ActivationFunctionType.Gelu,
                    scale=1.0 / WS,
                    bias=b1_sb[:, m : m + 1] if has_b1 else 0.0,
                )

        # ---- FFN2 mixed fp8-DR + fp16: per token half, 8 live accumulators ----
        for ihp in range(2):
            stps = [
                ps.tile([P, NT], f32, tag="stp", bufs=3, name=f"f2s{ihp}_{sl}")
                for sl in range(3)
            ]
            smps = [
                ps.tile([P, 512], f32, tag="smp", bufs=2, name=f"f2m{ihp}_{sl}")
                for sl in range(2)
            ]
            def accsl(il, dh):
                q = 2 * il + dh
                if q < 6:
                    return stps[q // 2][:, (q % 2) * 512 : (q % 2) * 512 + 384]
                return smps[q - 6][:, 0:384]
            # il-outer: ~40 consecutive matmuls per PSUM bank pair instead
            # of cycling all 8 banks every 8 matmuls (avoids the psum-queue
            # HAM oscillation), and lets each token chunk's residual start
            # while the next accumulates.
            NKH = MC - F2_8
            for il in range(4):
                tok = slice((4 * ihp + il) * P, (4 * ihp + il + 1) * P)
                for c in range(F2_8 // 2):
                    for dh_ in range(2):
                        nc.tensor.matmul(
                            accsl(il, dh_),
                            hT8[:, 2 * c : 2 * c + 2, tok],
                            w28p[:, 2 * c : 2 * c + 2, dh_ * 384 : (dh_ + 1) * 384],
                            start=(c == 0), stop=False,
                            perf_mode=DR,
                        )
                for kk in range(NKH):
                    for dh_ in range(2):
                        nc.tensor.matmul(
                            accsl(il, dh_),
                            hTh[:, kk, tok],
                            w2ht[kk][:, dh_ * 384 : (dh_ + 1) * 384],
                            start=False, stop=(kk == NKH - 1),
                        )
            for il in range(4):
                i = 4 * ihp + il
                for dh_ in range(2):
                    xsl = x_t[i][:, dh_ * 384 : (dh_ + 1) * 384]
                    nc.vector.scalar_tensor_tensor(
                        out=xsl, in0=accsl(il, dh_), scalar=1.0 / WS,
                        in1=xsl,
                        op0=mybir.AluOpType.mult, op1=mybir.AluOpType.add,
                    )
                    if has_b2:
                        nc.vector.tensor_add(
                            out=xsl, in0=xsl, in1=b2_bc[:, dh_ * 384 : (dh_ + 1) * 384]
                        )
                q = (nc.gpsimd, nc.sync, nc.scalar, nc.gpsimd)[il]
                q.dma_start(out=out_d[i * P : (i + 1) * P, :], in_=x_t[i])

        sb.release()
        ps.release()

    nc.compile()
    return nc


def _prep_inputs(inputs):
    """Host-side weight folding/reshaping. Returns (flags, common_map, x)."""
    x = np.ascontiguousarray(np.asarray(inputs["x"], dtype=np.float32))
    g1 = float(np.asarray(inputs["g1"]).reshape(-1)[0])
    be1 = float(np.asarray(inputs["be1"]).reshape(-1)[0])
    g2 = float(np.asarray(inputs["g2"]).reshape(-1)[0])
    be2 = float(np.asarray(inputs["be2"]).reshape(-1)[0])

    wq = np.asarray(inputs["wq"], np.float32)
    wk = np.asarray(inputs["wk"], np.float32)
    wv = np.asarray(inputs["wv"], np.float32)
    wo = np.asarray(inputs["wo"], np.float32)
    w1 = np.asarray(inputs["w1"], np.float32)
    w2 = np.asarray(inputs["w2"], np.float32)

    # LN affine folding (device LN computes (x - mean)/(std + eps) only)
    bq = np.asarray(inputs["bq"], np.float32) + be1 * wq.sum(axis=0)
    bk = np.asarray(inputs["bk"], np.float32) + be1 * wk.sum(axis=0)
    bv = np.asarray(inputs["bv"], np.float32) + be1 * wv.sum(axis=0)
    bo = np.asarray(inputs["bo"], np.float32)
    b1 = np.asarray(inputs["b1"], np.float32) + be2 * w1.sum(axis=0)
    b2 = np.asarray(inputs["b2"], np.float32)

    f8np = ml_dtypes.float8_e4m3

    def slab(w, scale, kchunks):
        # [d_in, m] -> [128, kchunks, m] slab layout with prescale (fp32)
        m = w.shape[1]
        return (w * scale).reshape(kchunks, P, m).transpose(1, 0, 2)

    w1s = slab(g2 * w1, WS, KC)     # [128, 6, F], x16
    w2s = slab(w2, WS, MC)          # [128, 24, D], x16

    common = {
        "wq8": np.ascontiguousarray(
            np.clip(slab(g1 * wq, WS, KC), -240, 240).astype(f8np)),
        "wk8": np.ascontiguousarray(
            np.clip(slab(g1 * wk, WS, KC), -240, 240).astype(f8np)),
        "wv8": np.ascontiguousarray(
            np.clip(slab(g1 * wv, WS, KC), -240, 240).astype(f8np)),
        "wo8": np.ascontiguousarray(
            np.clip(slab(wo, WS, KC), -240, 240).astype(f8np)),
        "w18": np.ascontiguousarray(
            np.clip(w1s[:, :F1_8], -240, 240).astype(f8np)),
        "w1h": np.ascontiguousarray(w1s[:, F1_8:].astype(np.float16)),
        "w28": np.ascontiguousarray(
            np.clip(w2s[:, :F2_8], -240, 240).astype(f8np)),
        "w2h": np.ascontiguousarray(w2s[:, F2_8:].astype(np.float16)),
    }
    flags = (
        bool(np.any(bq) or np.any(bk)),
        bool(np.any(bv)),
        bool(np.any(bo)),
        bool(np.any(b1)),
        bool(np.any(b2)),
    )
    has_bqk, has_bv, has_bo, has_b1, has_b2 = flags
    if has_bqk:
        common["bq"] = np.ascontiguousarray(WS * bq)
        common["bk"] = np.ascontiguousarray(WS * bk)
    if has_bv:
        common["bv"] = np.ascontiguousarray(WS * bv)
    if has_bo:
        common["bo"] = np.ascontiguousarray(bo)
    if has_b1:
        common["b1"] = np.ascontiguousarray(b1)
    if has_b2:
        common["b2"] = np.ascontiguousarray(b2)
    return flags, common, x


def kernel(**inputs):
    global LAST_RESULT
    flags, common, x = _prep_inputs(inputs)
    if flags not in _COMPILE_CACHE:
        _COMPILE_CACHE[flags] = _build(flags)
    nc = _COMPILE_CACHE[flags]

    n_cores = x.shape[0]
    in_maps = [dict(common, x=np.ascontiguousarray(x[i])) for i in range(n_cores)]
    trace = os.environ.get("BASS_KERNEL_TRACE") == "1"
    res = run_bass_kernel_spmd(nc, in_maps, list(range(n_cores)), trace=trace)
    LAST_RESULT = res
    out = np.stack([res.results[i]["out"] for i in range(n_cores)], axis=0)
    return out.astype(np.float32)
